# revision 40
# baseline (speedup 1.0000x reference)
"""Trainium2 Bass kernel for a GPT-2 style transformer block (v2, bf16).

Sharding (8 NeuronCores, SPMD-uniform program):
  - Tokens (B*S = 4096) sharded contiguously: core c owns tokens [512c, 512c+512).
  - Attention is head-sharded: core c computes heads {2c, 2c+1} over ALL tokens.
    AllToAlls exchange (Q^T, K^T) and V token-shards -> head-shards (split in two
    collectives so V compute overlaps the QK exchange), and O^T back.
  - All matmul operands are bf16 (fp32 PSUM accumulate); the residual stream
    stays fp32 in SBUF. LN scale/bias folded into following weights on host.
  - Weights live in SBUF slabs rotating through one shared pool tag so DMA
    prefetch of later phases overlaps earlier compute.
  - Attention K/V are SBUF-resident; scores are built transposed S^T[k, q], the
    exp output A^T feeds AV directly; lhsT = [ones | V_h] also produces softmax
    sums; normalization uses a PE broadcast matmul (no DRAM round-trip), with
    causal trimming of score/exp/AV free dims.
  - MLP runs in 8 fused blocks: fc (weight-stationary) -> gelu (one scalar
    activation instr, tanh approx) -> proj (h-stationary) accumulated into the
    fp32 residual in SBUF.
"""

import numpy as np

# ---------------------------------------------------------------- config

B, S, D, H = 2, 2048, 1024, 16
HD = D // H           # 64
FF = 4 * D            # 4096
NC = 8                # cores
TPC = B * S // NC     # 512 tokens per core
EPS = 1e-05

P = 128               # partitions
TT = TPC // P         # 4 token tiles per core
DK = D // P           # 8 contraction tiles over D
FFK = FF // P         # 32 tiles over FF
HPC = H // NC         # 2 heads per core
QB = TPC              # q-block width for attention (= shard width)
NQB = S // QB         # 4 q-blocks per batch
KPB = QB // P         # 4 k-tiles per q-block
NBLK = 8              # fused fc/proj blocks (512 ff features each)


def build_program():
    import contextlib

    import concourse.bass as bass
    import concourse.mybir as mybir
    import concourse.tile as tile
    from concourse import bacc
    from concourse.masks import make_identity, make_upper_triangular

    f32 = mybir.dt.float32
    f32r = mybir.dt.float32r
    bf16 = mybir.dt.bfloat16
    AF = mybir.ActivationFunctionType

    nc = bacc.Bacc("TRN2", target_bir_lowering=False, debug=False,
                   num_devices=NC)

    # ---- kernel I/O (per core) ----
    x_d = nc.dram_tensor("x", [TPC, D], f32, kind="ExternalInput").ap()
    caw_d = nc.dram_tensor("c_attn_w", [D, 3 * D], bf16, kind="ExternalInput").ap()
    cab_d = nc.dram_tensor("c_attn_b", [3 * D], f32, kind="ExternalInput").ap()
    cpw_d = nc.dram_tensor("c_proj_w", [D, D], bf16, kind="ExternalInput").ap()
    cpb_d = nc.dram_tensor("c_proj_b", [D], f32, kind="ExternalInput").ap()
    fcw_d = nc.dram_tensor("fc_w", [D, FF], bf16, kind="ExternalInput").ap()
    fcb_d = nc.dram_tensor("fc_b", [FF], f32, kind="ExternalInput").ap()
    pjw_d = nc.dram_tensor("proj_w", [FF, D], bf16, kind="ExternalInput").ap()
    pjb_d = nc.dram_tensor("proj_b", [D], f32, kind="ExternalInput").ap()
    out_d = nc.dram_tensor("out", [TPC, D], f32, kind="ExternalOutput").ap()

    PT = P * TPC  # elements in one [128, 512] slot region

    with tile.TileContext(nc) as tc:
        ctx = contextlib.ExitStack()
        with ctx:
            dram = ctx.enter_context(tc.tile_pool(name="dram", bufs=1,
                                                  space="DRAM"))
            consts = ctx.enter_context(tc.tile_pool(name="consts", bufs=1))
            wsl = ctx.enter_context(tc.tile_pool(name="wsl", bufs=9))
            resid = ctx.enter_context(tc.tile_pool(name="resid", bufs=1))
            acts = ctx.enter_context(tc.tile_pool(name="acts", bufs=1))
            attp = ctx.enter_context(tc.tile_pool(name="attp", bufs=1))
            temps = ctx.enter_context(tc.tile_pool(name="temps", bufs=4))
            stats = ctx.enter_context(tc.tile_pool(name="stats", bufs=2))
            psum = ctx.enter_context(tc.tile_pool(name="psum", bufs=1,
                                                  space="PSUM"))

            # a2a buffers (bf16); Q/K/V exchanged by three collectives, each
            # triggered as soon as its projection is evicted
            finQ = dram.tile([NC, PT], bf16)
            foutQ = dram.tile([NC, PT], bf16)
            finK = dram.tile([NC, PT], bf16)
            foutK = dram.tile([NC, PT], bf16)
            finV = dram.tile([NC, PT], bf16)
            foutV = dram.tile([NC, PT], bf16)
            a2a_bin = dram.tile([NC, PT], bf16)
            a2a_bout = dram.tile([NC, PT], bf16)

            # ---------------- x first: it heads the critical path ----------
            x_tiles = [resid.tile([P, D], f32, tag=f"x{t}", name=f"x{t}")
                       for t in range(TT)]
            xr = x_d.rearrange("(t p) d -> t p d", p=P)
            for t in range(TT):
                nc.sync.dma_start(x_tiles[t], xr[t])

            # ---------------- constants ----------------
            ident = consts.tile([P, P], bf16)
            make_identity(nc, ident)
            # mask[k, q] = 1 if q >= k (within a diagonal 128x128 strip)
            mask_f = consts.tile([P, P], f32)
            make_upper_triangular(nc, mask_f, val=1.0, diag=True)
            mask2 = consts.tile([P, HPC, P], bf16)
            for h in range(HPC):
                nc.vector.tensor_copy(out=mask2[:, h, :], in_=mask_f)

            ones_f = consts.tile([1, HD], f32)
            nc.vector.memset(ones_f, 1.0)
            ones64 = consts.tile([1, HD], f32r)
            nc.vector.tensor_copy(out=ones64, in_=ones_f)

            # per-partition bias tile, pre-transposed on host (a strided DMA
            # here would emit thousands of 4-byte descriptors)
            bias_d = nc.dram_tensor("bias_pp", [P, 2 * DK + FFK], f32,
                                    kind="ExternalInput").ap()
            bias_pp = consts.tile([P, 2 * DK + FFK], f32)
            nc.sync.dma_start(bias_pp, bias_d)
            cab_qk = bias_pp[:, 0:2 * DK]
            fcb_pp = bias_pp[:, 2 * DK:]

            def bcast_row(src_ap, off, n, name):
                t = consts.tile([P, n], f32, name=name)
                nc.sync.dma_start(t, bass.AP(
                    tensor=src_ap.tensor, offset=src_ap.offset + off,
                    ap=[[0, P], [1, n]]))
                return t

            # ---------------- weight slabs (shared rotating tag) ----------------
            # caw: 8 slabs [128, 3072]; cpw: 2 slabs [128, 4, 1024];
            # fcw: 8 m-slabs [128, 8, 512]; pjw: 8 slabs [128, 4, 1024].
            caw_r = caw_d.rearrange("(a p) f -> p a f", p=P)   # [128, 8, 3072]
            cpw_r = cpw_d.rearrange("(a p) d -> p a d", p=P)   # [128, 8, 1024]
            fcw_r = fcw_d.rearrange("(a p) f -> p a f", p=P)   # [128, 8, 4096]
            pjw_r = pjw_d.rearrange("(a p) d -> p a d", p=P)   # [128, 32, 1024]

            # c_attn weights column-grouped (Q slabs, then K, then V) so the
            # Q projection can start after only 2MB of weight DMA and each
            # group's collective fires as early as possible. The 2KB-slot tag
            # "w2k" is reused later by the MLP hidden tiles.
            caw_g = []
            for g in range(3):
                grp = []
                for k in range(DK):
                    t = wsl.tile([P, D], bf16, tag="w2k", bufs=24,
                                 name=f"caw{g}_{k}")
                    nc.gpsimd.dma_start(t, caw_r[:, k, g * D:(g + 1) * D])
                    grp.append(t)
                caw_g.append(grp)
            cawQ_sb, cawK_sb, cawV_sb = caw_g
            # cpw/fcw/pjw slab DMAs are emitted AFTER the forward collectives
            # on their queues so the collectives are not stuck behind their
            # slot-waits / descriptor generation.

            # ---------------- phase 1: LN1 + transpose ----------------
            def ln_transpose(dstT):
                """LayerNorm (ddof=1, eps on std) each [P, D] token tile of the
                residual, then PE-transpose into dstT [P, DK, TPC] (bf16)."""
                for t in range(TT):
                    xt = x_tiles[t]
                    st = stats.tile([P, 2, nc.vector.BN_STATS_DIM], f32,
                                    tag="bnst")
                    xg = xt.rearrange("p (g d) -> p g d", g=2)
                    for g in range(2):
                        nc.vector.bn_stats(out=st[:, g, :], in_=xg[:, g, :])
                    mv = stats.tile([P, nc.vector.BN_AGGR_DIM], f32, tag="mv")
                    nc.vector.bn_aggr(out=mv, in_=st)
                    sdev = stats.tile([P, 1], f32, tag="sdev")
                    nc.scalar.activation(out=sdev, in_=mv[:, 1:2], func=AF.Sqrt,
                                         scale=float(D) / (D - 1))
                    nc.vector.tensor_scalar_add(sdev, sdev, EPS)
                    rstd = stats.tile([P, 1], f32, tag="rstd")
                    nc.vector.reciprocal(out=rstd, in_=sdev)
                    nmr = stats.tile([P, 1], f32, tag="nmr")
                    nc.vector.tensor_scalar(out=nmr, in0=mv[:, 0:1],
                                            scalar1=rstd, scalar2=-1.0,
                                            op0=mybir.AluOpType.mult,
                                            op1=mybir.AluOpType.mult)
                    xn = temps.tile([P, D], bf16, tag="xn", bufs=2)
                    nc.scalar.activation(out=xn, in_=xt, func=AF.Identity,
                                         bias=nmr, scale=rstd)
                    for g in range(2):  # 4 transposes batched per eviction
                        pt = psum.tile([P, 4 * P], bf16, tag="ps", bufs=4,
                                       name="tp")
                        for i in range(4):
                            d = 4 * g + i
                            nc.tensor.transpose(pt[:, i * P:(i + 1) * P],
                                                xn[:, d * P:(d + 1) * P],
                                                ident)
                        nc.vector.tensor_copy(
                            out=dstT[:, 4 * g:4 * g + 4, t * P:(t + 1) * P],
                            in_=pt.rearrange("p (i c) -> p i c", c=P))

            xnT = acts.tile([P, DK, TPC], bf16, tag="xnT", name="xnT")
            ln_transpose(xnT)

            # ---------------- phase 2a: Q then K projections ----------------
            for half, (wsb, fin, fout) in enumerate(
                    [(cawQ_sb, finQ, foutQ), (cawK_sb, finK, foutK)]):
                for j in range(DK):  # 8 feature tiles each
                    m = half * DK + j
                    ps = psum.tile([P, TPC], f32, tag="ps", bufs=4,
                                   name="mmps")
                    for k in range(DK):
                        nc.tensor.matmul(ps, wsb[k][:, j * P:(j + 1) * P],
                                         xnT[:, k, :],
                                         start=(k == 0), stop=(k == DK - 1))
                    sb = temps.tile([P, TPC], bf16, tag="ev", bufs=3,
                                    name="ev")
                    nc.scalar.activation(out=sb, in_=ps, func=AF.Identity,
                                         bias=cab_qk[:, m:m + 1])
                    nc.sync.dma_start(
                        fin[j, :].rearrange("(a b) -> a b", b=TPC), sb)
                nc.gpsimd.collective_compute(
                    "AllToAll", mybir.AluOpType.bypass,
                    replica_groups=[list(range(NC))],
                    ins=[fin.opt()], outs=[fout.opt()])

            # ---------------- phase 2b: V projection -> finV ----------------
            vb_bc = bcast_row(cab_d, 2 * D, D, "vb_bc")    # V bias along feats
            for t in range(TT):
                ps = psum.tile([P, D], f32, tag="ps", bufs=4, name="mmps")
                for nb in range(2):
                    ns = D // 2
                    for k in range(DK):
                        nc.tensor.matmul(
                            ps[:, nb * ns:(nb + 1) * ns],
                            xnT[:, k, t * P:(t + 1) * P],
                            cawV_sb[k][:, nb * ns:(nb + 1) * ns],
                            start=(k == 0), stop=(k == DK - 1))
                vt = temps.tile([P, D], bf16, tag="vt", bufs=1, name="vt")
                nc.vector.tensor_add(out=vt, in0=ps, in1=vb_bc)
                # one DMA covering all 8 destination slots
                nc.sync.dma_start(
                    finV[:, t * P * P:(t + 1) * P * P].rearrange(
                        "j (p f) -> p j f", p=P),
                    vt.rearrange("p (j f) -> p j f", f=P))

            # ---------------- phase 3b: forward AllToAll (V) ----------------
            nc.gpsimd.collective_compute(
                "AllToAll", mybir.AluOpType.bypass,
                replica_groups=[list(range(NC))],
                ins=[finV.opt()], outs=[foutV.opt()])

            # cpw/fcw slabs: issued on gpsimd after the forward collectives
            # (slots are free once QKV released the caw slabs; the bwd
            # collective behind them is not needed until attention ends).
            cpw_sb = []
            for c in range(2):
                t = wsl.tile([P, 4, D], bf16, tag="wsl", name=f"cpw{c}")
                nc.gpsimd.dma_start(t, cpw_r[:, 4 * c:4 * c + 4, :])
                cpw_sb.append(t)
            fcw_sb = []
            for g in range(NBLK):
                t = wsl.tile([P, DK, 512], bf16, tag="wsl", name=f"fcw{g}")
                nc.gpsimd.dma_start(t, fcw_r[:, :, g * 512:(g + 1) * 512])
                fcw_sb.append(t)

            # ---------------- phase 4: attention (my 2 heads, all tokens) ----
            # K^T resident: [128 feat, slot, 512 tok]
            ktr = attp.tile([P, NC, TPC], bf16, tag="ktr", name="ktr")
            nc.sync.dma_start(
                ktr, foutK.rearrange("n (p t) -> p n t", p=P))
            # V resident with ones col per head: [128 tok, slot, t, 2*(64+1)]
            # foutV lands contiguously in vtmp (big DMA descriptors); a vector
            # repack inserts the per-head layout (tiny DMA descriptors would
            # cost ~25us otherwise).
            vres = attp.tile([P, NC, TT, HPC * (HD + 1)], bf16, tag="vres",
                             name="vres")
            for j in range(NC):
                nc.vector.memset(
                    vres[:, j, :, :].rearrange("p t (h c) -> p t h c",
                                               c=HD + 1)[:, :, :, HD:HD + 1],
                    1.0)
                vtmp = attp.tile([P, TT, P], bf16, tag="vtmp", bufs=1,
                                 name="vtmp")
                nc.sync.dma_start(
                    vtmp, foutV[j, :].rearrange("(t p f) -> p t f", t=TT, p=P))
                for t in range(TT):
                    nc.vector.tensor_copy(
                        out=vres[:, j, t, :].rearrange(
                            "p (h c) -> p h c", c=HD + 1)[:, :, 0:HD],
                        in_=vtmp[:, t, :].rearrange("p (h c) -> p h c", c=HD))

            isc = 1.0 / float(np.sqrt(HD))
            for b in range(B):
                for qb in range(NQB):
                    slot_q = b * NQB + qb
                    qt = attp.tile([P, QB], bf16, tag="qt", bufs=2, name="qt")
                    nc.sync.dma_start(
                        qt, foutQ[slot_q, :].rearrange("(a b) -> a b",
                                                           b=TPC))
                    op = psum.tile([HD + 1, HPC * QB], f32, tag="ps", bufs=4,
                                   name="op")
                    nkt = (qb + 1) * KPB
                    for kt in range(nkt):
                        src_slot = b * NQB + kt // KPB
                        off = (kt % KPB) * P
                        d = kt - qb * KPB  # >= 0 on diagonal strips
                        qs = max(d, 0) * P
                        sps = psum.tile([P, HPC * QB], f32, tag="ps", bufs=4,
                                        name="sps")
                        for h in range(HPC):
                            nc.tensor.matmul(
                                sps[:, h * QB + qs:(h + 1) * QB],
                                ktr[h * HD:(h + 1) * HD, src_slot,
                                    off:off + P],
                                qt[h * HD:(h + 1) * HD, qs:QB],
                                start=True, stop=True)
                        at = attp.tile([P, HPC, QB], bf16, tag="at", bufs=3,
                                       name="at")
                        spsv = sps.rearrange("p (h q) -> p h q", h=HPC)
                        nc.scalar.activation(out=at[:, :, qs:QB],
                                             in_=spsv[:, :, qs:QB],
                                             func=AF.Exp, scale=isc)
                        if d >= 0:
                            nc.vector.tensor_mul(out=at[:, :, qs:qs + P],
                                                 in0=at[:, :, qs:qs + P],
                                                 in1=mask2)
                        for h in range(HPC):
                            nc.tensor.matmul(
                                op[:, h * QB + qs:(h + 1) * QB],
                                vres[:, src_slot, kt % KPB,
                                     h * (HD + 1):(h + 1) * (HD + 1)],
                                at[:, h, qs:QB],
                                start=(kt == 0), stop=(kt == nkt - 1))
                    # normalize: recip of the sums row, partition-broadcast
                    # via a cheap PE matmul (ones64^T @ rc), O^T copied to
                    # SBUF so the multiply has a single PSUM operand
                    rc0 = attp.tile([1, HPC * QB], f32, tag="rc", bufs=2,
                                    name="rc0")
                    nc.vector.reciprocal_approx_fast(out=rc0,
                                                     in_=op[HD:HD + 1, :])
                    rc = attp.tile([1, HPC * QB], f32r, tag="rc", bufs=2,
                                   name="rc")
                    nc.gpsimd.tensor_copy(out=rc, in_=rc0)
                    otv = attp.tile([HD, HPC * QB], bf16, tag="otv", bufs=2,
                                    name="otv")
                    nc.vector.tensor_copy(out=otv, in_=op[0:HD, :])
                    for h in range(HPC):
                        bc = psum.tile([HD, QB], f32, tag="ps", bufs=4,
                                       name="bc")
                        nc.tensor.matmul(bc, ones64,
                                         rc[:, h * QB:(h + 1) * QB],
                                         start=True, stop=True)
                        nc.vector.tensor_mul(
                            out=otv[:, h * QB:(h + 1) * QB],
                            in0=otv[:, h * QB:(h + 1) * QB], in1=bc)
                        nc.sync.dma_start(
                            a2a_bin[slot_q, h * HD * TPC:(h + 1) * HD * TPC]
                            .rearrange("(a b) -> a b", b=TPC),
                            otv[:, h * QB:(h + 1) * QB])

            # ---------------- phase 3c: backward AllToAll (O^T) ----------------
            nc.gpsimd.collective_compute(
                "AllToAll", mybir.AluOpType.bypass,
                replica_groups=[list(range(NC))],
                ins=[a2a_bin.opt()], outs=[a2a_bout.opt()])

            # ---------------- phase 5: attn out-proj + residual (in place) ----
            otf = acts.tile([P, DK, TPC], bf16, tag="xnT", name="otf")
            nc.sync.dma_start(otf, a2a_bout[:].rearrange(
                "n (p t) -> p n t", p=P))
            # pjw slabs on the sync queue here: the only sync work emitted
            # after them is the final out DMA, so their slot-waits (on caw/cpw/
            # fcw slab releases, all PE-driven) cannot head-of-line block any
            # compute. Slabs 0-4 bind to slots already free by this point;
            # 5-7 bind as the first fc blocks release their fcw slabs.
            pjw_sb = []
            for g in range(NBLK):
                t = wsl.tile([P, 4, D], bf16, tag="wsl", name=f"pjw{g}")
                nc.sync.dma_start(t, pjw_r[:, 4 * g:4 * g + 4, :])
                pjw_sb.append(t)
            cpb_bc = bcast_row(cpb_d, 0, D, "cpb_bc")
            pjb_bc = bcast_row(pjb_d, 0, D, "pjb_bc")
            for t in range(TT):
                nc.vector.tensor_add(out=x_tiles[t], in0=x_tiles[t],
                                     in1=cpb_bc)
            for t in range(TT):
                ps = psum.tile([P, D], f32, tag="ps", bufs=4, name="mmps")
                for nb in range(2):
                    ns = D // 2
                    for k in range(DK):
                        nc.tensor.matmul(
                            ps[:, nb * ns:(nb + 1) * ns],
                            otf[:, k, t * P:(t + 1) * P],
                            cpw_sb[k // 4][:, k % 4, nb * ns:(nb + 1) * ns],
                            start=(k == 0), stop=(k == DK - 1))
                nc.vector.tensor_add(out=x_tiles[t], in0=ps, in1=x_tiles[t])

            # ---------------- phase 6: LN2 + transpose ----------------
            xn2T = acts.tile([P, DK, TPC], bf16, tag="xnT", name="xn2T")
            ln_transpose(xn2T)

            # ---------------- phase 7+8: fused MLP blocks ----------------
            for t in range(TT):
                nc.vector.tensor_add(out=x_tiles[t], in0=x_tiles[t],
                                     in1=pjb_bc)
            for g in range(NBLK):
                ht = []
                for mm in range(4):
                    m = g * 4 + mm
                    ps = psum.tile([P, TPC], f32, tag="ps", bufs=4,
                                   name="mmps")
                    for k in range(DK):
                        nc.tensor.matmul(
                            ps, fcw_sb[g][:, k, mm * P:(mm + 1) * P],
                            xn2T[:, k, :],
                            start=(k == 0), stop=(k == DK - 1))
                    hm = wsl.tile([P, TPC], bf16, tag="w2k", bufs=24,
                                  name="hT")
                    nc.scalar.activation(out=hm, in_=ps,
                                         func=AF.Gelu_apprx_tanh,
                                         bias=fcb_pp[:, m:m + 1])
                    ht.append(hm)
                for t in range(TT):
                    ps = psum.tile([P, D], f32, tag="ps", bufs=4,
                                   name="mmps")
                    for nb in range(2):
                        ns = D // 2
                        for kk in range(4):
                            nc.tensor.matmul(
                                ps[:, nb * ns:(nb + 1) * ns],
                                ht[kk][:, t * P:(t + 1) * P],
                                pjw_sb[g][:, kk, nb * ns:(nb + 1) * ns],
                                start=(kk == 0), stop=(kk == 3))
                    nc.vector.tensor_add(out=x_tiles[t], in0=ps,
                                         in1=x_tiles[t])

            # ---------------- output ----------------
            outr = out_d.rearrange("(t p) d -> t p d", p=P)
            for t in range(TT):
                nc.sync.dma_start(outr[t], x_tiles[t])

    nc.compile()
    return nc


_NC_CACHE = None


def _get_program():
    global _NC_CACHE
    if _NC_CACHE is None:
        _NC_CACHE = build_program()
    return _NC_CACHE


def host_fold(inputs):
    """Fold LN scale/bias into the following matmul weights; cast weights to
    bf16 (host side)."""
    import ml_dtypes
    bf = ml_dtypes.bfloat16

    def f(a):
        return np.ascontiguousarray(np.asarray(a), dtype=np.float32)
    x = f(inputs["x"]).reshape(B * S, D)
    caw0 = f(inputs["c_attn_w"])
    fcw0 = f(inputs["fc_w"])
    caw = caw0 * f(inputs["ln1_w"])[:, None]
    cab = f(inputs["c_attn_b"]) + f(inputs["ln1_b"]) @ caw0
    fcw = fcw0 * f(inputs["ln2_w"])[:, None]
    fcb = f(inputs["fc_b"]) + f(inputs["ln2_b"]) @ fcw0

    def w(a):
        return np.ascontiguousarray(np.asarray(a, dtype=bf))
    # per-partition bias tile [128, 48]: QK biases then fc biases, transposed
    bias_pp = np.concatenate([cab[:2 * D].reshape(2 * DK, P).T,
                              fcb.reshape(FFK, P).T], axis=1)
    return {
        "x": x,
        "c_attn_w": w(caw), "c_attn_b": f(cab),
        "c_proj_w": w(inputs["c_proj_w"]), "c_proj_b": f(inputs["c_proj_b"]),
        "fc_w": w(fcw), "fc_b": f(fcb),
        "proj_w": w(inputs["proj_w"]), "proj_b": f(inputs["proj_b"]),
        "bias_pp": np.ascontiguousarray(bias_pp, dtype=np.float32),
    }


def make_in_maps(inputs):
    full = host_fold(inputs)
    in_maps = []
    for c in range(NC):
        m = dict(full)
        m["x"] = np.ascontiguousarray(full["x"][c * TPC:(c + 1) * TPC])
        in_maps.append(m)
    return in_maps


def kernel(**inputs) -> np.ndarray:
    from concourse import bass_utils
    nc = _get_program()
    in_maps = make_in_maps(inputs)
    res = bass_utils.run_bass_kernel_spmd(nc, in_maps, core_ids=list(range(NC)))
    out = np.concatenate([res.results[c]["out"] for c in range(NC)], axis=0)
    return out.reshape(B, S, D)


# revision 42
# speedup vs baseline: 1.1879x; 1.1879x over previous
"""Trainium2 Bass kernel for a GPT-2 style transformer block (v2, bf16).

Sharding (8 NeuronCores, SPMD-uniform program):
  - Tokens (B*S = 4096) sharded contiguously: core c owns tokens [512c, 512c+512).
  - Attention is head-sharded: core c computes heads {2c, 2c+1} over ALL tokens.
    AllToAlls exchange (Q^T, K^T) and V token-shards -> head-shards (split in two
    collectives so V compute overlaps the QK exchange), and O^T back.
  - All matmul operands are bf16 (fp32 PSUM accumulate); the residual stream
    stays fp32 in SBUF. LN scale/bias folded into following weights on host.
  - Weights live in SBUF slabs rotating through one shared pool tag so DMA
    prefetch of later phases overlaps earlier compute.
  - Attention K/V are SBUF-resident; scores are built transposed S^T[k, q], the
    exp output A^T feeds AV directly; lhsT = [ones | V_h] also produces softmax
    sums; normalization uses a PE broadcast matmul (no DRAM round-trip), with
    causal trimming of score/exp/AV free dims.
  - MLP runs in 8 fused blocks: fc (weight-stationary) -> gelu (one scalar
    activation instr, tanh approx) -> proj (h-stationary) accumulated into the
    fp32 residual in SBUF.
"""

import numpy as np

# ---------------------------------------------------------------- config

B, S, D, H = 2, 2048, 1024, 16
HD = D // H           # 64
FF = 4 * D            # 4096
NC = 8                # cores
TPC = B * S // NC     # 512 tokens per core
EPS = 1e-05

P = 128               # partitions
TT = TPC // P         # 4 token tiles per core
DK = D // P           # 8 contraction tiles over D
FFK = FF // P         # 32 tiles over FF
HPC = H // NC         # 2 heads per core
QB = TPC              # q-block width for attention (= shard width)
NQB = S // QB         # 4 q-blocks per batch
KPB = QB // P         # 4 k-tiles per q-block
NBLK = 8              # fused fc/proj blocks (512 ff features each)


def build_program():
    import contextlib

    import concourse.bass as bass
    import concourse.mybir as mybir
    import concourse.tile as tile
    from concourse import bacc
    from concourse.masks import make_identity, make_upper_triangular

    f32 = mybir.dt.float32
    f32r = mybir.dt.float32r
    bf16 = mybir.dt.bfloat16
    AF = mybir.ActivationFunctionType

    nc = bacc.Bacc("TRN2", target_bir_lowering=False, debug=False,
                   num_devices=NC)

    # ---- kernel I/O (per core) ----
    x_d = nc.dram_tensor("x", [TPC, D], f32, kind="ExternalInput").ap()
    caw_d = nc.dram_tensor("c_attn_w", [D, 3 * D], bf16, kind="ExternalInput").ap()
    cab_d = nc.dram_tensor("c_attn_b", [3 * D], f32, kind="ExternalInput").ap()
    cpw_d = nc.dram_tensor("c_proj_w", [D, D], bf16, kind="ExternalInput").ap()
    cpb_d = nc.dram_tensor("c_proj_b", [D], f32, kind="ExternalInput").ap()
    fcw_d = nc.dram_tensor("fc_w", [D, FF], bf16, kind="ExternalInput").ap()
    fcb_d = nc.dram_tensor("fc_b", [FF], f32, kind="ExternalInput").ap()
    pjw_d = nc.dram_tensor("proj_w", [FF, D], bf16, kind="ExternalInput").ap()
    pjb_d = nc.dram_tensor("proj_b", [D], f32, kind="ExternalInput").ap()
    out_d = nc.dram_tensor("out", [TPC, D], f32, kind="ExternalOutput").ap()

    PT = P * TPC  # elements in one [128, 512] slot region

    with tile.TileContext(nc) as tc:
        ctx = contextlib.ExitStack()
        with ctx:
            dram = ctx.enter_context(tc.tile_pool(name="dram", bufs=1,
                                                  space="DRAM"))
            consts = ctx.enter_context(tc.tile_pool(name="consts", bufs=1))
            wsl = ctx.enter_context(tc.tile_pool(name="wsl", bufs=9))
            resid = ctx.enter_context(tc.tile_pool(name="resid", bufs=1))
            acts = ctx.enter_context(tc.tile_pool(name="acts", bufs=1))
            attp = ctx.enter_context(tc.tile_pool(name="attp", bufs=1))
            temps = ctx.enter_context(tc.tile_pool(name="temps", bufs=4))
            stats = ctx.enter_context(tc.tile_pool(name="stats", bufs=2))
            psum = ctx.enter_context(tc.tile_pool(name="psum", bufs=1,
                                                  space="PSUM"))

            # a2a buffers (bf16); Q/K/V exchanged by three collectives, each
            # triggered as soon as its projection is evicted
            finQ = dram.tile([NC, PT], bf16)
            foutQ = dram.tile([NC, PT], bf16)
            finK = dram.tile([NC, PT], bf16)
            foutK = dram.tile([NC, PT], bf16)
            finV = dram.tile([NC, PT], bf16)
            foutV = dram.tile([NC, PT], bf16)
            a2a_bin = dram.tile([NC, PT], bf16)
            a2a_bout = dram.tile([NC, PT], bf16)

            # ---------------- x first: it heads the critical path ----------
            x_tiles = [resid.tile([P, D], f32, tag=f"x{t}", name=f"x{t}")
                       for t in range(TT)]
            xr = x_d.rearrange("(t p) d -> t p d", p=P)
            for t in range(TT):
                nc.sync.dma_start(x_tiles[t], xr[t])

            # ---------------- constants ----------------
            ident = consts.tile([P, P], bf16)
            make_identity(nc, ident)
            # mask[k, q] = 1 if q >= k (within a diagonal 128x128 strip)
            mask_f = consts.tile([P, P], f32)
            make_upper_triangular(nc, mask_f, val=1.0, diag=True)
            mask2 = consts.tile([P, HPC, P], bf16)
            for h in range(HPC):
                nc.vector.tensor_copy(out=mask2[:, h, :], in_=mask_f)

            ones_f = consts.tile([1, HD], f32)
            nc.vector.memset(ones_f, 1.0)
            ones64 = consts.tile([1, HD], f32r)
            nc.vector.tensor_copy(out=ones64, in_=ones_f)

            # per-partition bias tile, pre-transposed on host (a strided DMA
            # here would emit thousands of 4-byte descriptors)
            bias_d = nc.dram_tensor("bias_pp", [P, 2 * DK + FFK], f32,
                                    kind="ExternalInput").ap()
            bias_pp = consts.tile([P, 2 * DK + FFK], f32)
            nc.sync.dma_start(bias_pp, bias_d)
            cab_qk = bias_pp[:, 0:2 * DK]
            fcb_pp = bias_pp[:, 2 * DK:]

            def bcast_row(src_ap, off, n, name):
                t = consts.tile([P, n], f32, name=name)
                nc.sync.dma_start(t, bass.AP(
                    tensor=src_ap.tensor, offset=src_ap.offset + off,
                    ap=[[0, P], [1, n]]))
                return t

            # ---------------- weight slabs (shared rotating tag) ----------------
            # caw: 8 slabs [128, 3072]; cpw: 2 slabs [128, 4, 1024];
            # fcw: 8 m-slabs [128, 8, 512]; pjw: 8 slabs [128, 4, 1024].
            caw_r = caw_d.rearrange("(a p) f -> p a f", p=P)   # [128, 8, 3072]
            cpw_r = cpw_d.rearrange("(a p) d -> p a d", p=P)   # [128, 8, 1024]
            fcw_r = fcw_d.rearrange("(a p) f -> p a f", p=P)   # [128, 8, 4096]
            pjw_r = pjw_d.rearrange("(a p) d -> p a d", p=P)   # [128, 32, 1024]

            # c_attn weights column-grouped (Q slabs, then K, then V) so the
            # Q projection can start after only 2MB of weight DMA and each
            # group's collective fires as early as possible. The 2KB-slot tag
            # "w2k" is reused later by the MLP hidden tiles.
            caw_g = []
            for g in range(3):
                grp = []
                for k in range(DK):
                    t = wsl.tile([P, D], bf16, tag="w2k", bufs=24,
                                 name=f"caw{g}_{k}")
                    nc.gpsimd.dma_start(t, caw_r[:, k, g * D:(g + 1) * D])
                    grp.append(t)
                caw_g.append(grp)
            cawQ_sb, cawK_sb, cawV_sb = caw_g
            # cpw/fcw/pjw slab DMAs are emitted AFTER the forward collectives
            # on their queues so the collectives are not stuck behind their
            # slot-waits / descriptor generation.

            # ---------------- phase 1: LN1 + transpose ----------------
            def ln_transpose(dstT):
                """LayerNorm (ddof=1, eps on std) each [P, D] token tile of the
                residual, then PE-transpose into dstT [P, DK, TPC] (bf16)."""
                for t in range(TT):
                    xt = x_tiles[t]
                    st = stats.tile([P, 2, nc.vector.BN_STATS_DIM], f32,
                                    tag="bnst")
                    xg = xt.rearrange("p (g d) -> p g d", g=2)
                    for g in range(2):
                        nc.vector.bn_stats(out=st[:, g, :], in_=xg[:, g, :])
                    mv = stats.tile([P, nc.vector.BN_AGGR_DIM], f32, tag="mv")
                    nc.vector.bn_aggr(out=mv, in_=st)
                    sdev = stats.tile([P, 1], f32, tag="sdev")
                    nc.scalar.activation(out=sdev, in_=mv[:, 1:2], func=AF.Sqrt,
                                         scale=float(D) / (D - 1))
                    nc.vector.tensor_scalar_add(sdev, sdev, EPS)
                    rstd = stats.tile([P, 1], f32, tag="rstd")
                    nc.vector.reciprocal(out=rstd, in_=sdev)
                    nmr = stats.tile([P, 1], f32, tag="nmr")
                    nc.vector.tensor_scalar(out=nmr, in0=mv[:, 0:1],
                                            scalar1=rstd, scalar2=-1.0,
                                            op0=mybir.AluOpType.mult,
                                            op1=mybir.AluOpType.mult)
                    xn = temps.tile([P, D], bf16, tag="xn", bufs=2)
                    nc.scalar.activation(out=xn, in_=xt, func=AF.Identity,
                                         bias=nmr, scale=rstd)
                    for g in range(2):  # 4 transposes batched per eviction
                        pt = psum.tile([P, 4 * P], bf16, tag="ps", bufs=4,
                                       name="tp")
                        for i in range(4):
                            d = 4 * g + i
                            nc.tensor.transpose(pt[:, i * P:(i + 1) * P],
                                                xn[:, d * P:(d + 1) * P],
                                                ident)
                        nc.vector.tensor_copy(
                            out=dstT[:, 4 * g:4 * g + 4, t * P:(t + 1) * P],
                            in_=pt.rearrange("p (i c) -> p i c", c=P))

            xnT = acts.tile([P, DK, TPC], bf16, tag="xnT", name="xnT")
            ln_transpose(xnT)

            # ---------------- phase 2a: Q then K projections ----------------
            for half, (wsb, fin, fout) in enumerate(
                    [(cawQ_sb, finQ, foutQ), (cawK_sb, finK, foutK)]):
                for j in range(DK):  # 8 feature tiles each
                    m = half * DK + j
                    ps = psum.tile([P, TPC], f32, tag="ps", bufs=4,
                                   name="mmps")
                    for k in range(DK):
                        nc.tensor.matmul(ps, wsb[k][:, j * P:(j + 1) * P],
                                         xnT[:, k, :],
                                         start=(k == 0), stop=(k == DK - 1))
                    sb = temps.tile([P, TPC], bf16, tag="ev", bufs=3,
                                    name="ev")
                    nc.scalar.activation(out=sb, in_=ps, func=AF.Identity,
                                         bias=cab_qk[:, m:m + 1])
                    nc.sync.dma_start(
                        fin[j, :].rearrange("(a b) -> a b", b=TPC), sb)
                nc.gpsimd.collective_compute(
                    "AllToAll", mybir.AluOpType.bypass,
                    replica_groups=[list(range(NC))],
                    ins=[fin.opt()], outs=[fout.opt()])

            # ---------------- phase 2b: V projection -> finV ----------------
            vb_bc = bcast_row(cab_d, 2 * D, D, "vb_bc")    # V bias along feats
            for t in range(TT):
                ps = psum.tile([P, D], f32, tag="ps", bufs=4, name="mmps")
                for nb in range(2):
                    ns = D // 2
                    for k in range(DK):
                        nc.tensor.matmul(
                            ps[:, nb * ns:(nb + 1) * ns],
                            xnT[:, k, t * P:(t + 1) * P],
                            cawV_sb[k][:, nb * ns:(nb + 1) * ns],
                            start=(k == 0), stop=(k == DK - 1))
                vt = temps.tile([P, D], bf16, tag="vt", bufs=1, name="vt")
                nc.vector.tensor_add(out=vt, in0=ps, in1=vb_bc)
                # one DMA covering all 8 destination slots
                nc.sync.dma_start(
                    finV[:, t * P * P:(t + 1) * P * P].rearrange(
                        "j (p f) -> p j f", p=P),
                    vt.rearrange("p (j f) -> p j f", f=P))

            # ---------------- phase 3b: forward AllToAll (V) ----------------
            nc.gpsimd.collective_compute(
                "AllToAll", mybir.AluOpType.bypass,
                replica_groups=[list(range(NC))],
                ins=[finV.opt()], outs=[foutV.opt()])

            # cpw/fcw slabs: issued on gpsimd after the forward collectives
            # (slots are free once QKV released the caw slabs; the bwd
            # collective behind them is not needed until attention ends).
            cpw_sb = []
            for c in range(2):
                t = wsl.tile([P, 4, D], bf16, tag="wsl", name=f"cpw{c}")
                nc.gpsimd.dma_start(t, cpw_r[:, 4 * c:4 * c + 4, :])
                cpw_sb.append(t)
            fcw_sb = []
            for g in range(NBLK):
                t = wsl.tile([P, DK, 512], bf16, tag="wsl", name=f"fcw{g}")
                nc.gpsimd.dma_start(t, fcw_r[:, :, g * 512:(g + 1) * 512])
                fcw_sb.append(t)

            # ---------------- phase 4: attention (my 2 heads, all tokens) ----
            # K^T resident: [128 feat, slot, 512 tok]
            ktr = attp.tile([P, NC, TPC], bf16, tag="ktr", name="ktr")
            nc.sync.dma_start(
                ktr, foutK.rearrange("n (p t) -> p n t", p=P))
            # V resident with ones col per head: [128 tok, slot, t, 2*(64+1)]
            # foutV lands contiguously in vtmp (big DMA descriptors); a vector
            # repack inserts the per-head layout (tiny DMA descriptors would
            # cost ~25us otherwise).
            vres = attp.tile([P, NC, TT, HPC * (HD + 1)], bf16, tag="vres",
                             name="vres")
            for j in range(NC):
                nc.vector.memset(
                    vres[:, j, :, :].rearrange("p t (h c) -> p t h c",
                                               c=HD + 1)[:, :, :, HD:HD + 1],
                    1.0)
                vtmp = attp.tile([P, TT, P], bf16, tag="vtmp", bufs=1,
                                 name="vtmp")
                nc.sync.dma_start(
                    vtmp, foutV[j, :].rearrange("(t p f) -> p t f", t=TT, p=P))
                for t in range(TT):
                    nc.vector.tensor_copy(
                        out=vres[:, j, t, :].rearrange(
                            "p (h c) -> p h c", c=HD + 1)[:, :, 0:HD],
                        in_=vtmp[:, t, :].rearrange("p (h c) -> p h c", c=HD))

            isc = 1.0 / float(np.sqrt(HD))
            for b in range(B):
                for qb in range(NQB):
                    slot_q = b * NQB + qb
                    qt = attp.tile([P, QB], bf16, tag="qt", bufs=2, name="qt")
                    nc.sync.dma_start(
                        qt, foutQ[slot_q, :].rearrange("(a b) -> a b",
                                                           b=TPC))
                    op = psum.tile([HD + 1, HPC * QB], f32, tag="ps", bufs=4,
                                   name="op")
                    nkt = (qb + 1) * KPB
                    for kt in range(nkt):
                        src_slot = b * NQB + kt // KPB
                        off = (kt % KPB) * P
                        d = kt - qb * KPB  # >= 0 on diagonal strips
                        qs = max(d, 0) * P
                        sps = psum.tile([P, HPC * QB], f32, tag="ps", bufs=4,
                                        name="sps")
                        for h in range(HPC):
                            nc.tensor.matmul(
                                sps[:, h * QB + qs:(h + 1) * QB],
                                ktr[h * HD:(h + 1) * HD, src_slot,
                                    off:off + P],
                                qt[h * HD:(h + 1) * HD, qs:QB],
                                start=True, stop=True)
                        at = attp.tile([P, HPC, QB], bf16, tag="at", bufs=3,
                                       name="at")
                        spsv = sps.rearrange("p (h q) -> p h q", h=HPC)
                        nc.scalar.activation(out=at[:, :, qs:QB],
                                             in_=spsv[:, :, qs:QB],
                                             func=AF.Exp, scale=isc)
                        if d >= 0:
                            nc.vector.tensor_mul(out=at[:, :, qs:qs + P],
                                                 in0=at[:, :, qs:qs + P],
                                                 in1=mask2)
                        for h in range(HPC):
                            nc.tensor.matmul(
                                op[:, h * QB + qs:(h + 1) * QB],
                                vres[:, src_slot, kt % KPB,
                                     h * (HD + 1):(h + 1) * (HD + 1)],
                                at[:, h, qs:QB],
                                start=(kt == 0), stop=(kt == nkt - 1))
                    # normalize: recip of the sums row, partition-broadcast
                    # via a cheap PE matmul (ones64^T @ rc), O^T copied to
                    # SBUF so the multiply has a single PSUM operand
                    # 1/s = exp(-ln(s)) on the scalar engine: both funcs live
                    # in one act table, and it keeps the q-block tail off the
                    # vector queue (vector.reciprocal on [1,1024] costs 6.5us)
                    lns = attp.tile([1, HPC * QB], f32, tag="rc", bufs=2,
                                    name="lns")
                    nc.scalar.activation(out=lns, in_=op[HD:HD + 1, :],
                                         func=AF.Ln)
                    rc = attp.tile([1, HPC * QB], f32r, tag="rc", bufs=2,
                                   name="rc")
                    nc.scalar.activation(out=rc, in_=lns, func=AF.Exp,
                                         scale=-1.0)
                    otv = attp.tile([HD, HPC * QB], bf16, tag="otv", bufs=2,
                                    name="otv")
                    nc.vector.tensor_copy(out=otv, in_=op[0:HD, :])
                    for h in range(HPC):
                        bc = psum.tile([HD, QB], f32, tag="ps", bufs=4,
                                       name="bc")
                        nc.tensor.matmul(bc, ones64,
                                         rc[:, h * QB:(h + 1) * QB],
                                         start=True, stop=True)
                        nc.vector.tensor_mul(
                            out=otv[:, h * QB:(h + 1) * QB],
                            in0=otv[:, h * QB:(h + 1) * QB], in1=bc)
                        nc.sync.dma_start(
                            a2a_bin[slot_q, h * HD * TPC:(h + 1) * HD * TPC]
                            .rearrange("(a b) -> a b", b=TPC),
                            otv[:, h * QB:(h + 1) * QB])

            # ---------------- phase 3c: backward AllToAll (O^T) ----------------
            nc.gpsimd.collective_compute(
                "AllToAll", mybir.AluOpType.bypass,
                replica_groups=[list(range(NC))],
                ins=[a2a_bin.opt()], outs=[a2a_bout.opt()])

            # ---------------- phase 5: attn out-proj + residual (in place) ----
            otf = acts.tile([P, DK, TPC], bf16, tag="xnT", name="otf")
            nc.sync.dma_start(otf, a2a_bout[:].rearrange(
                "n (p t) -> p n t", p=P))
            # pjw slabs on the sync queue here: the only sync work emitted
            # after them is the final out DMA, so their slot-waits (on caw/cpw/
            # fcw slab releases, all PE-driven) cannot head-of-line block any
            # compute. Slabs 0-4 bind to slots already free by this point;
            # 5-7 bind as the first fc blocks release their fcw slabs.
            pjw_sb = []
            for g in range(NBLK):
                t = wsl.tile([P, 4, D], bf16, tag="wsl", name=f"pjw{g}")
                nc.sync.dma_start(t, pjw_r[:, 4 * g:4 * g + 4, :])
                pjw_sb.append(t)
            cpb_bc = bcast_row(cpb_d, 0, D, "cpb_bc")
            pjb_bc = bcast_row(pjb_d, 0, D, "pjb_bc")
            for t in range(TT):
                nc.vector.tensor_add(out=x_tiles[t], in0=x_tiles[t],
                                     in1=cpb_bc)
            for t in range(TT):
                ps = psum.tile([P, D], f32, tag="ps", bufs=4, name="mmps")
                for nb in range(2):
                    ns = D // 2
                    for k in range(DK):
                        nc.tensor.matmul(
                            ps[:, nb * ns:(nb + 1) * ns],
                            otf[:, k, t * P:(t + 1) * P],
                            cpw_sb[k // 4][:, k % 4, nb * ns:(nb + 1) * ns],
                            start=(k == 0), stop=(k == DK - 1))
                nc.vector.tensor_add(out=x_tiles[t], in0=ps, in1=x_tiles[t])

            # ---------------- phase 6: LN2 + transpose ----------------
            xn2T = acts.tile([P, DK, TPC], bf16, tag="xnT", name="xn2T")
            ln_transpose(xn2T)

            # ---------------- phase 7+8: fused MLP blocks ----------------
            for t in range(TT):
                nc.vector.tensor_add(out=x_tiles[t], in0=x_tiles[t],
                                     in1=pjb_bc)
            for g in range(NBLK):
                ht = []
                for mm in range(4):
                    m = g * 4 + mm
                    ps = psum.tile([P, TPC], f32, tag="ps", bufs=4,
                                   name="mmps")
                    for k in range(DK):
                        nc.tensor.matmul(
                            ps, fcw_sb[g][:, k, mm * P:(mm + 1) * P],
                            xn2T[:, k, :],
                            start=(k == 0), stop=(k == DK - 1))
                    hm = wsl.tile([P, TPC], bf16, tag="w2k", bufs=24,
                                  name="hT")
                    nc.scalar.activation(out=hm, in_=ps,
                                         func=AF.Gelu_apprx_tanh,
                                         bias=fcb_pp[:, m:m + 1])
                    ht.append(hm)
                for t in range(TT):
                    ps = psum.tile([P, D], f32, tag="ps", bufs=4,
                                   name="mmps")
                    for nb in range(2):
                        ns = D // 2
                        for kk in range(4):
                            nc.tensor.matmul(
                                ps[:, nb * ns:(nb + 1) * ns],
                                ht[kk][:, t * P:(t + 1) * P],
                                pjw_sb[g][:, kk, nb * ns:(nb + 1) * ns],
                                start=(kk == 0), stop=(kk == 3))
                    nc.vector.tensor_add(out=x_tiles[t], in0=ps,
                                         in1=x_tiles[t])

            # ---------------- output ----------------
            outr = out_d.rearrange("(t p) d -> t p d", p=P)
            for t in range(TT):
                nc.sync.dma_start(outr[t], x_tiles[t])

    nc.compile()
    return nc


_NC_CACHE = None


def _get_program():
    global _NC_CACHE
    if _NC_CACHE is None:
        _NC_CACHE = build_program()
    return _NC_CACHE


def host_fold(inputs):
    """Fold LN scale/bias into the following matmul weights; cast weights to
    bf16 (host side)."""
    import ml_dtypes
    bf = ml_dtypes.bfloat16

    def f(a):
        return np.ascontiguousarray(np.asarray(a), dtype=np.float32)
    x = f(inputs["x"]).reshape(B * S, D)
    caw0 = f(inputs["c_attn_w"])
    fcw0 = f(inputs["fc_w"])
    caw = caw0 * f(inputs["ln1_w"])[:, None]
    cab = f(inputs["c_attn_b"]) + f(inputs["ln1_b"]) @ caw0
    fcw = fcw0 * f(inputs["ln2_w"])[:, None]
    fcb = f(inputs["fc_b"]) + f(inputs["ln2_b"]) @ fcw0

    def w(a):
        return np.ascontiguousarray(np.asarray(a, dtype=bf))
    # per-partition bias tile [128, 48]: QK biases then fc biases, transposed
    bias_pp = np.concatenate([cab[:2 * D].reshape(2 * DK, P).T,
                              fcb.reshape(FFK, P).T], axis=1)
    return {
        "x": x,
        "c_attn_w": w(caw), "c_attn_b": f(cab),
        "c_proj_w": w(inputs["c_proj_w"]), "c_proj_b": f(inputs["c_proj_b"]),
        "fc_w": w(fcw), "fc_b": f(fcb),
        "proj_w": w(inputs["proj_w"]), "proj_b": f(inputs["proj_b"]),
        "bias_pp": np.ascontiguousarray(bias_pp, dtype=np.float32),
    }


def make_in_maps(inputs):
    full = host_fold(inputs)
    in_maps = []
    for c in range(NC):
        m = dict(full)
        m["x"] = np.ascontiguousarray(full["x"][c * TPC:(c + 1) * TPC])
        in_maps.append(m)
    return in_maps


def kernel(**inputs) -> np.ndarray:
    from concourse import bass_utils
    nc = _get_program()
    in_maps = make_in_maps(inputs)
    res = bass_utils.run_bass_kernel_spmd(nc, in_maps, core_ids=list(range(NC)))
    out = np.concatenate([res.results[c]["out"] for c in range(NC)], axis=0)
    return out.reshape(B, S, D)


# revision 44
# speedup vs baseline: 1.2317x; 1.0369x over previous
"""Trainium2 Bass kernel for a GPT-2 style transformer block (v2, bf16).

Sharding (8 NeuronCores, SPMD-uniform program):
  - Tokens (B*S = 4096) sharded contiguously: core c owns tokens [512c, 512c+512).
  - Attention is head-sharded: core c computes heads {2c, 2c+1} over ALL tokens.
    AllToAlls exchange (Q^T, K^T) and V token-shards -> head-shards (split in two
    collectives so V compute overlaps the QK exchange), and O^T back.
  - All matmul operands are bf16 (fp32 PSUM accumulate); the residual stream
    stays fp32 in SBUF. LN scale/bias folded into following weights on host.
  - Weights live in SBUF slabs rotating through one shared pool tag so DMA
    prefetch of later phases overlaps earlier compute.
  - Attention K/V are SBUF-resident; scores are built transposed S^T[k, q], the
    exp output A^T feeds AV directly; lhsT = [ones | V_h] also produces softmax
    sums; normalization uses a PE broadcast matmul (no DRAM round-trip), with
    causal trimming of score/exp/AV free dims.
  - MLP runs in 8 fused blocks: fc (weight-stationary) -> gelu (one scalar
    activation instr, tanh approx) -> proj (h-stationary) accumulated into the
    fp32 residual in SBUF.
"""

import numpy as np

# ---------------------------------------------------------------- config

B, S, D, H = 2, 2048, 1024, 16
HD = D // H           # 64
FF = 4 * D            # 4096
NC = 8                # cores
TPC = B * S // NC     # 512 tokens per core
EPS = 1e-05

P = 128               # partitions
TT = TPC // P         # 4 token tiles per core
DK = D // P           # 8 contraction tiles over D
FFK = FF // P         # 32 tiles over FF
HPC = H // NC         # 2 heads per core
QB = TPC              # q-block width for attention (= shard width)
NQB = S // QB         # 4 q-blocks per batch
KPB = QB // P         # 4 k-tiles per q-block
NBLK = 8              # fused fc/proj blocks (512 ff features each)


def build_program():
    import contextlib

    import concourse.bass as bass
    import concourse.mybir as mybir
    import concourse.tile as tile
    from concourse import bacc
    from concourse.masks import make_identity, make_upper_triangular

    f32 = mybir.dt.float32
    f32r = mybir.dt.float32r
    bf16 = mybir.dt.bfloat16
    AF = mybir.ActivationFunctionType

    nc = bacc.Bacc("TRN2", target_bir_lowering=False, debug=False,
                   num_devices=NC)

    # ---- kernel I/O (per core) ----
    x_d = nc.dram_tensor("x", [TPC, D], f32, kind="ExternalInput").ap()
    caw_d = nc.dram_tensor("c_attn_w", [D, 3 * D], bf16, kind="ExternalInput").ap()
    cab_d = nc.dram_tensor("c_attn_b", [3 * D], f32, kind="ExternalInput").ap()
    cpw_d = nc.dram_tensor("c_proj_w", [D, D], bf16, kind="ExternalInput").ap()
    cpb_d = nc.dram_tensor("c_proj_b", [D], f32, kind="ExternalInput").ap()
    fcw_d = nc.dram_tensor("fc_w", [D, FF], bf16, kind="ExternalInput").ap()
    fcb_d = nc.dram_tensor("fc_b", [FF], f32, kind="ExternalInput").ap()
    pjw_d = nc.dram_tensor("proj_w", [FF, D], bf16, kind="ExternalInput").ap()
    pjb_d = nc.dram_tensor("proj_b", [D], f32, kind="ExternalInput").ap()
    out_d = nc.dram_tensor("out", [TPC, D], f32, kind="ExternalOutput").ap()

    PT = P * TPC  # elements in one [128, 512] slot region

    with tile.TileContext(nc) as tc:
        ctx = contextlib.ExitStack()
        with ctx:
            dram = ctx.enter_context(tc.tile_pool(name="dram", bufs=1,
                                                  space="DRAM"))
            consts = ctx.enter_context(tc.tile_pool(name="consts", bufs=1))
            wsl = ctx.enter_context(tc.tile_pool(name="wsl", bufs=9))
            resid = ctx.enter_context(tc.tile_pool(name="resid", bufs=1))
            acts = ctx.enter_context(tc.tile_pool(name="acts", bufs=1))
            attp = ctx.enter_context(tc.tile_pool(name="attp", bufs=1))
            temps = ctx.enter_context(tc.tile_pool(name="temps", bufs=4))
            stats = ctx.enter_context(tc.tile_pool(name="stats", bufs=2))
            psum = ctx.enter_context(tc.tile_pool(name="psum", bufs=1,
                                                  space="PSUM"))

            # a2a buffers (bf16); Q/K/V exchanged by three collectives, each
            # triggered as soon as its projection is evicted
            finQ = dram.tile([NC, PT], bf16)
            foutQ = dram.tile([NC, PT], bf16)
            finK = dram.tile([NC, PT], bf16)
            foutK = dram.tile([NC, PT], bf16)
            finV = dram.tile([NC, PT], bf16)
            foutV = dram.tile([NC, PT], bf16)
            a2a_bin = dram.tile([NC, PT], bf16)
            a2a_bout = dram.tile([NC, PT], bf16)

            # ---------------- x first: it heads the critical path ----------
            x_tiles = [resid.tile([P, D], f32, tag=f"x{t}", name=f"x{t}")
                       for t in range(TT)]
            xr = x_d.rearrange("(t p) d -> t p d", p=P)
            for t in range(TT):
                nc.sync.dma_start(x_tiles[t], xr[t])

            # ---------------- constants ----------------
            ident = consts.tile([P, P], bf16)
            make_identity(nc, ident)
            # mask[k, q] = 1 if q >= k (within a diagonal 128x128 strip)
            mask_f = consts.tile([P, P], f32)
            make_upper_triangular(nc, mask_f, val=1.0, diag=True)
            mask2 = consts.tile([P, HPC, P], bf16)
            for h in range(HPC):
                nc.vector.tensor_copy(out=mask2[:, h, :], in_=mask_f)

            # per-partition bias tile, pre-transposed on host (a strided DMA
            # here would emit thousands of 4-byte descriptors)
            bias_d = nc.dram_tensor("bias_pp", [P, 2 * DK + FFK], f32,
                                    kind="ExternalInput").ap()
            bias_pp = consts.tile([P, 2 * DK + FFK], f32)
            nc.sync.dma_start(bias_pp, bias_d)
            cab_qk = bias_pp[:, 0:2 * DK]
            fcb_pp = bias_pp[:, 2 * DK:]

            def bcast_row(src_ap, off, n, name):
                t = consts.tile([P, n], f32, name=name)
                nc.sync.dma_start(t, bass.AP(
                    tensor=src_ap.tensor, offset=src_ap.offset + off,
                    ap=[[0, P], [1, n]]))
                return t

            # ---------------- weight slabs (shared rotating tag) ----------------
            # caw: 8 slabs [128, 3072]; cpw: 2 slabs [128, 4, 1024];
            # fcw: 8 m-slabs [128, 8, 512]; pjw: 8 slabs [128, 4, 1024].
            caw_r = caw_d.rearrange("(a p) f -> p a f", p=P)   # [128, 8, 3072]
            cpw_r = cpw_d.rearrange("(a p) d -> p a d", p=P)   # [128, 8, 1024]
            fcw_r = fcw_d.rearrange("(a p) f -> p a f", p=P)   # [128, 8, 4096]
            pjw_r = pjw_d.rearrange("(a p) d -> p a d", p=P)   # [128, 32, 1024]

            # c_attn weights column-grouped (Q slabs, then K, then V) so the
            # Q projection can start after only 2MB of weight DMA and each
            # group's collective fires as early as possible. The 2KB-slot tag
            # "w2k" is reused later by the MLP hidden tiles.
            caw_g = []
            for g in range(3):
                grp = []
                for k in range(DK):
                    t = wsl.tile([P, D], bf16, tag="w2k", bufs=24,
                                 name=f"caw{g}_{k}")
                    nc.gpsimd.dma_start(t, caw_r[:, k, g * D:(g + 1) * D])
                    grp.append(t)
                caw_g.append(grp)
            cawQ_sb, cawK_sb, cawV_sb = caw_g
            # cpw/fcw/pjw slab DMAs are emitted AFTER the forward collectives
            # on their queues so the collectives are not stuck behind their
            # slot-waits / descriptor generation.

            # ---------------- phase 1: LN1 + transpose ----------------
            def ln_transpose(dstT):
                """LayerNorm (ddof=1, eps on std) each [P, D] token tile of the
                residual, then PE-transpose into dstT [P, DK, TPC] (bf16)."""
                for t in range(TT):
                    xt = x_tiles[t]
                    st = stats.tile([P, 2, nc.vector.BN_STATS_DIM], f32,
                                    tag="bnst")
                    xg = xt.rearrange("p (g d) -> p g d", g=2)
                    for g in range(2):
                        nc.vector.bn_stats(out=st[:, g, :], in_=xg[:, g, :])
                    mv = stats.tile([P, nc.vector.BN_AGGR_DIM], f32, tag="mv")
                    nc.vector.bn_aggr(out=mv, in_=st)
                    sdev = stats.tile([P, 1], f32, tag="sdev")
                    nc.scalar.activation(out=sdev, in_=mv[:, 1:2], func=AF.Sqrt,
                                         scale=float(D) / (D - 1))
                    nc.vector.tensor_scalar_add(sdev, sdev, EPS)
                    rstd = stats.tile([P, 1], f32, tag="rstd")
                    nc.vector.reciprocal(out=rstd, in_=sdev)
                    nmr = stats.tile([P, 1], f32, tag="nmr")
                    nc.vector.tensor_scalar(out=nmr, in0=mv[:, 0:1],
                                            scalar1=rstd, scalar2=-1.0,
                                            op0=mybir.AluOpType.mult,
                                            op1=mybir.AluOpType.mult)
                    xn = temps.tile([P, D], bf16, tag="xn", bufs=2)
                    nc.scalar.activation(out=xn, in_=xt, func=AF.Identity,
                                         bias=nmr, scale=rstd)
                    for g in range(2):  # 4 transposes batched per eviction
                        pt = psum.tile([P, 4 * P], bf16, tag="ps", bufs=4,
                                       name="tp")
                        for i in range(4):
                            d = 4 * g + i
                            nc.tensor.transpose(pt[:, i * P:(i + 1) * P],
                                                xn[:, d * P:(d + 1) * P],
                                                ident)
                        nc.vector.tensor_copy(
                            out=dstT[:, 4 * g:4 * g + 4, t * P:(t + 1) * P],
                            in_=pt.rearrange("p (i c) -> p i c", c=P))

            xnT = acts.tile([P, DK, TPC], bf16, tag="xnT", name="xnT")
            ln_transpose(xnT)

            # ---------------- phase 2a: Q then K projections ----------------
            for half, (wsb, fin, fout) in enumerate(
                    [(cawQ_sb, finQ, foutQ), (cawK_sb, finK, foutK)]):
                for j in range(DK):  # 8 feature tiles each
                    m = half * DK + j
                    ps = psum.tile([P, TPC], f32, tag="ps", bufs=4,
                                   name="mmps")
                    for k in range(DK):
                        nc.tensor.matmul(ps, wsb[k][:, j * P:(j + 1) * P],
                                         xnT[:, k, :],
                                         start=(k == 0), stop=(k == DK - 1))
                    sb = temps.tile([P, TPC], bf16, tag="ev", bufs=2,
                                    name="ev")
                    nc.scalar.activation(out=sb, in_=ps, func=AF.Identity,
                                         bias=cab_qk[:, m:m + 1])
                    nc.sync.dma_start(
                        fin[j, :].rearrange("(a b) -> a b", b=TPC), sb)
                nc.gpsimd.collective_compute(
                    "AllToAll", mybir.AluOpType.bypass,
                    replica_groups=[list(range(NC))],
                    ins=[fin.opt()], outs=[fout.opt()])

            # ---------------- phase 2b: V projection -> finV ----------------
            vb_bc = bcast_row(cab_d, 2 * D, D, "vb_bc")    # V bias along feats
            for t in range(TT):
                ps = psum.tile([P, D], f32, tag="ps", bufs=4, name="mmps")
                for nb in range(2):
                    ns = D // 2
                    for k in range(DK):
                        nc.tensor.matmul(
                            ps[:, nb * ns:(nb + 1) * ns],
                            xnT[:, k, t * P:(t + 1) * P],
                            cawV_sb[k][:, nb * ns:(nb + 1) * ns],
                            start=(k == 0), stop=(k == DK - 1))
                vt = temps.tile([P, D], bf16, tag="vt", bufs=1, name="vt")
                nc.vector.tensor_add(out=vt, in0=ps, in1=vb_bc)
                # one DMA covering all 8 destination slots
                nc.sync.dma_start(
                    finV[:, t * P * P:(t + 1) * P * P].rearrange(
                        "j (p f) -> p j f", p=P),
                    vt.rearrange("p (j f) -> p j f", f=P))

            # ---------------- phase 3b: forward AllToAll (V) ----------------
            nc.gpsimd.collective_compute(
                "AllToAll", mybir.AluOpType.bypass,
                replica_groups=[list(range(NC))],
                ins=[finV.opt()], outs=[foutV.opt()])

            # cpw/fcw slabs: issued on gpsimd after the forward collectives
            # (slots are free once QKV released the caw slabs; the bwd
            # collective behind them is not needed until attention ends).
            cpw_sb = []
            for c in range(2):
                t = wsl.tile([P, 4, D], bf16, tag="wsl", name=f"cpw{c}")
                nc.gpsimd.dma_start(t, cpw_r[:, 4 * c:4 * c + 4, :])
                cpw_sb.append(t)
            fcw_sb = []
            for g in range(NBLK):
                t = wsl.tile([P, DK, 512], bf16, tag="wsl", name=f"fcw{g}")
                nc.gpsimd.dma_start(t, fcw_r[:, :, g * 512:(g + 1) * 512])
                fcw_sb.append(t)

            # ---------------- phase 4: attention (my 2 heads, all tokens) ----
            # K^T resident: [128 feat, slot, 512 tok]
            ktr = attp.tile([P, NC, TPC], bf16, tag="ktr", name="ktr")
            nc.sync.dma_start(
                ktr, foutK.rearrange("n (p t) -> p n t", p=P))
            # V resident with ones col per head: [128 tok, slot, t, 2*(64+1)]
            # foutV lands contiguously in vtmp (big DMA descriptors); a vector
            # repack inserts the per-head layout (tiny DMA descriptors would
            # cost ~25us otherwise).
            vres = attp.tile([P, NC, TT, HPC * (HD + 1)], bf16, tag="vres",
                             name="vres")
            for j in range(NC):
                nc.vector.memset(
                    vres[:, j, :, :].rearrange("p t (h c) -> p t h c",
                                               c=HD + 1)[:, :, :, HD:HD + 1],
                    1.0)
                vtmp = attp.tile([P, TT, P], bf16, tag="vtmp", bufs=1,
                                 name="vtmp")
                nc.sync.dma_start(
                    vtmp, foutV[j, :].rearrange("(t p f) -> p t f", t=TT, p=P))
                for t in range(TT):
                    nc.vector.tensor_copy(
                        out=vres[:, j, t, :].rearrange(
                            "p (h c) -> p h c", c=HD + 1)[:, :, 0:HD],
                        in_=vtmp[:, t, :].rearrange("p (h c) -> p h c", c=HD))

            isc = 1.0 / float(np.sqrt(HD))
            for b in range(B):
                for qb in range(NQB):
                    slot_q = b * NQB + qb
                    qt = attp.tile([P, QB], bf16, tag="qt", bufs=2, name="qt")
                    nc.sync.dma_start(
                        qt, foutQ[slot_q, :].rearrange("(a b) -> a b",
                                                           b=TPC))
                    op = psum.tile([HD + 1, HPC * QB], f32, tag="ps", bufs=4,
                                   name="op")
                    nkt = (qb + 1) * KPB
                    for kt in range(nkt):
                        src_slot = b * NQB + kt // KPB
                        off = (kt % KPB) * P
                        d = kt - qb * KPB  # >= 0 on diagonal strips
                        qs = max(d, 0) * P
                        sps = psum.tile([P, HPC * QB], f32, tag="ps", bufs=4,
                                        name="sps")
                        for h in range(HPC):
                            nc.tensor.matmul(
                                sps[:, h * QB + qs:(h + 1) * QB],
                                ktr[h * HD:(h + 1) * HD, src_slot,
                                    off:off + P],
                                qt[h * HD:(h + 1) * HD, qs:QB],
                                start=True, stop=True)
                        at = attp.tile([P, HPC, QB], bf16, tag="at", bufs=3,
                                       name="at")
                        spsv = sps.rearrange("p (h q) -> p h q", h=HPC)
                        nc.scalar.activation(out=at[:, :, qs:QB],
                                             in_=spsv[:, :, qs:QB],
                                             func=AF.Exp, scale=isc)
                        if d >= 0:
                            nc.vector.tensor_mul(out=at[:, :, qs:qs + P],
                                                 in0=at[:, :, qs:qs + P],
                                                 in1=mask2)
                        for h in range(HPC):
                            nc.tensor.matmul(
                                op[:, h * QB + qs:(h + 1) * QB],
                                vres[:, src_slot, kt % KPB,
                                     h * (HD + 1):(h + 1) * (HD + 1)],
                                at[:, h, qs:QB],
                                start=(kt == 0), stop=(kt == nkt - 1))
                    # normalize: recip of the sums row, partition-broadcast
                    # via a cheap PE matmul (ones64^T @ rc), O^T copied to
                    # SBUF so the multiply has a single PSUM operand
                    # 1/s = exp(-ln(s)) on the scalar engine: both funcs live
                    # in one act table, and it keeps the q-block tail off the
                    # vector queue (vector.reciprocal on [1,1024] costs 6.5us)
                    lns = attp.tile([1, HPC * QB], f32, tag="rc", bufs=2,
                                    name="lns")
                    nc.scalar.activation(out=lns, in_=op[HD:HD + 1, :],
                                         func=AF.Ln)
                    rc = attp.tile([1, HPC * QB], f32, tag="rc", bufs=2,
                                   name="rc")
                    nc.scalar.activation(out=rc, in_=lns, func=AF.Exp,
                                         scale=-1.0)
                    # partition-broadcast 1/s via a DRAM bounce; the PE queue
                    # stays free so the next q-block's matmuls run under this
                    rcd = dram.tile([HPC * QB], f32, tag="rcd", name="rcd",
                                    bufs=2)
                    nc.sync.dma_start(rcd, rc)
                    rbc = attp.tile([HD, HPC * QB], f32, tag="rbc", bufs=1,
                                    name="rbc")
                    nc.sync.dma_start(rbc, bass.AP(
                        tensor=rcd.tensor, offset=rcd.offset,
                        ap=[[0, HD], [1, HPC * QB]]))
                    otv = attp.tile([HD, HPC * QB], bf16, tag="otv", bufs=2,
                                    name="otv")
                    nc.vector.tensor_mul(out=otv, in0=op[0:HD, :], in1=rbc)
                    for h in range(HPC):
                        nc.sync.dma_start(
                            a2a_bin[slot_q, h * HD * TPC:(h + 1) * HD * TPC]
                            .rearrange("(a b) -> a b", b=TPC),
                            otv[:, h * QB:(h + 1) * QB])

            # ---------------- phase 3c: backward AllToAll (O^T) ----------------
            nc.gpsimd.collective_compute(
                "AllToAll", mybir.AluOpType.bypass,
                replica_groups=[list(range(NC))],
                ins=[a2a_bin.opt()], outs=[a2a_bout.opt()])

            # ---------------- phase 5: attn out-proj + residual (in place) ----
            otf = acts.tile([P, DK, TPC], bf16, tag="xnT", name="otf")
            nc.sync.dma_start(otf, a2a_bout[:].rearrange(
                "n (p t) -> p n t", p=P))
            # pjw slabs on the sync queue here: the only sync work emitted
            # after them is the final out DMA, so their slot-waits (on caw/cpw/
            # fcw slab releases, all PE-driven) cannot head-of-line block any
            # compute. Slabs 0-4 bind to slots already free by this point;
            # 5-7 bind as the first fc blocks release their fcw slabs.
            pjw_sb = []
            for g in range(NBLK):
                t = wsl.tile([P, 4, D], bf16, tag="wsl", name=f"pjw{g}")
                nc.sync.dma_start(t, pjw_r[:, 4 * g:4 * g + 4, :])
                pjw_sb.append(t)
            cpb_bc = bcast_row(cpb_d, 0, D, "cpb_bc")
            pjb_bc = bcast_row(pjb_d, 0, D, "pjb_bc")
            for t in range(TT):
                nc.vector.tensor_add(out=x_tiles[t], in0=x_tiles[t],
                                     in1=cpb_bc)
            for t in range(TT):
                ps = psum.tile([P, D], f32, tag="ps", bufs=4, name="mmps")
                for nb in range(2):
                    ns = D // 2
                    for k in range(DK):
                        nc.tensor.matmul(
                            ps[:, nb * ns:(nb + 1) * ns],
                            otf[:, k, t * P:(t + 1) * P],
                            cpw_sb[k // 4][:, k % 4, nb * ns:(nb + 1) * ns],
                            start=(k == 0), stop=(k == DK - 1))
                nc.vector.tensor_add(out=x_tiles[t], in0=ps, in1=x_tiles[t])

            # ---------------- phase 6: LN2 + transpose ----------------
            xn2T = acts.tile([P, DK, TPC], bf16, tag="xnT", name="xn2T")
            ln_transpose(xn2T)

            # ---------------- phase 7+8: fused MLP blocks ----------------
            for t in range(TT):
                nc.vector.tensor_add(out=x_tiles[t], in0=x_tiles[t],
                                     in1=pjb_bc)
            for g in range(NBLK):
                ht = []
                for mm in range(4):
                    m = g * 4 + mm
                    ps = psum.tile([P, TPC], f32, tag="ps", bufs=4,
                                   name="mmps")
                    for k in range(DK):
                        nc.tensor.matmul(
                            ps, fcw_sb[g][:, k, mm * P:(mm + 1) * P],
                            xn2T[:, k, :],
                            start=(k == 0), stop=(k == DK - 1))
                    hm = wsl.tile([P, TPC], bf16, tag="w2k", bufs=24,
                                  name="hT")
                    nc.scalar.activation(out=hm, in_=ps,
                                         func=AF.Gelu_apprx_tanh,
                                         bias=fcb_pp[:, m:m + 1])
                    ht.append(hm)
                for t in range(TT):
                    ps = psum.tile([P, D], f32, tag="ps", bufs=4,
                                   name="mmps")
                    for nb in range(2):
                        ns = D // 2
                        for kk in range(4):
                            nc.tensor.matmul(
                                ps[:, nb * ns:(nb + 1) * ns],
                                ht[kk][:, t * P:(t + 1) * P],
                                pjw_sb[g][:, kk, nb * ns:(nb + 1) * ns],
                                start=(kk == 0), stop=(kk == 3))
                    nc.vector.tensor_add(out=x_tiles[t], in0=ps,
                                         in1=x_tiles[t])

            # ---------------- output ----------------
            outr = out_d.rearrange("(t p) d -> t p d", p=P)
            for t in range(TT):
                nc.sync.dma_start(outr[t], x_tiles[t])

    nc.compile()
    return nc


_NC_CACHE = None


def _get_program():
    global _NC_CACHE
    if _NC_CACHE is None:
        _NC_CACHE = build_program()
    return _NC_CACHE


def host_fold(inputs):
    """Fold LN scale/bias into the following matmul weights; cast weights to
    bf16 (host side)."""
    import ml_dtypes
    bf = ml_dtypes.bfloat16

    def f(a):
        return np.ascontiguousarray(np.asarray(a), dtype=np.float32)
    x = f(inputs["x"]).reshape(B * S, D)
    caw0 = f(inputs["c_attn_w"])
    fcw0 = f(inputs["fc_w"])
    caw = caw0 * f(inputs["ln1_w"])[:, None]
    cab = f(inputs["c_attn_b"]) + f(inputs["ln1_b"]) @ caw0
    fcw = fcw0 * f(inputs["ln2_w"])[:, None]
    fcb = f(inputs["fc_b"]) + f(inputs["ln2_b"]) @ fcw0

    def w(a):
        return np.ascontiguousarray(np.asarray(a, dtype=bf))
    # per-partition bias tile [128, 48]: QK biases then fc biases, transposed
    bias_pp = np.concatenate([cab[:2 * D].reshape(2 * DK, P).T,
                              fcb.reshape(FFK, P).T], axis=1)
    return {
        "x": x,
        "c_attn_w": w(caw), "c_attn_b": f(cab),
        "c_proj_w": w(inputs["c_proj_w"]), "c_proj_b": f(inputs["c_proj_b"]),
        "fc_w": w(fcw), "fc_b": f(fcb),
        "proj_w": w(inputs["proj_w"]), "proj_b": f(inputs["proj_b"]),
        "bias_pp": np.ascontiguousarray(bias_pp, dtype=np.float32),
    }


def make_in_maps(inputs):
    full = host_fold(inputs)
    in_maps = []
    for c in range(NC):
        m = dict(full)
        m["x"] = np.ascontiguousarray(full["x"][c * TPC:(c + 1) * TPC])
        in_maps.append(m)
    return in_maps


def kernel(**inputs) -> np.ndarray:
    from concourse import bass_utils
    nc = _get_program()
    in_maps = make_in_maps(inputs)
    res = bass_utils.run_bass_kernel_spmd(nc, in_maps, core_ids=list(range(NC)))
    out = np.concatenate([res.results[c]["out"] for c in range(NC)], axis=0)
    return out.reshape(B, S, D)


# revision 48
# speedup vs baseline: 1.2606x; 1.0234x over previous
"""Trainium2 Bass kernel for a GPT-2 style transformer block (v2, bf16).

Sharding (8 NeuronCores, SPMD-uniform program):
  - Tokens (B*S = 4096) sharded contiguously: core c owns tokens [512c, 512c+512).
  - Attention is head-sharded: core c computes heads {2c, 2c+1} over ALL tokens.
    AllToAlls exchange (Q^T, K^T) and V token-shards -> head-shards (split in two
    collectives so V compute overlaps the QK exchange), and O^T back.
  - All matmul operands are bf16 (fp32 PSUM accumulate); the residual stream
    stays fp32 in SBUF. LN scale/bias folded into following weights on host.
  - Weights live in SBUF slabs rotating through one shared pool tag so DMA
    prefetch of later phases overlaps earlier compute.
  - Attention K/V are SBUF-resident; scores are built transposed S^T[k, q], the
    exp output A^T feeds AV directly; lhsT = [ones | V_h] also produces softmax
    sums; normalization uses a PE broadcast matmul (no DRAM round-trip), with
    causal trimming of score/exp/AV free dims.
  - MLP runs in 8 fused blocks: fc (weight-stationary) -> gelu (one scalar
    activation instr, tanh approx) -> proj (h-stationary) accumulated into the
    fp32 residual in SBUF.
"""

import numpy as np

# ---------------------------------------------------------------- config

B, S, D, H = 2, 2048, 1024, 16
HD = D // H           # 64
FF = 4 * D            # 4096
NC = 8                # cores
TPC = B * S // NC     # 512 tokens per core
EPS = 1e-05

P = 128               # partitions
TT = TPC // P         # 4 token tiles per core
DK = D // P           # 8 contraction tiles over D
FFK = FF // P         # 32 tiles over FF
HPC = H // NC         # 2 heads per core
QB = TPC              # q-block width for attention (= shard width)
NQB = S // QB         # 4 q-blocks per batch
KPB = QB // P         # 4 k-tiles per q-block
NBLK = 8              # fused fc/proj blocks (512 ff features each)


def build_program():
    import contextlib

    import concourse.bass as bass
    import concourse.mybir as mybir
    import concourse.tile as tile
    from concourse import bacc
    from concourse.masks import make_identity, make_upper_triangular

    f32 = mybir.dt.float32
    f32r = mybir.dt.float32r
    bf16 = mybir.dt.bfloat16
    AF = mybir.ActivationFunctionType

    nc = bacc.Bacc("TRN2", target_bir_lowering=False, debug=False,
                   num_devices=NC)

    # ---- kernel I/O (per core) ----
    x_d = nc.dram_tensor("x", [TPC, D], f32, kind="ExternalInput").ap()
    caw_d = nc.dram_tensor("c_attn_w", [D, 3 * D], bf16, kind="ExternalInput").ap()
    cab_d = nc.dram_tensor("c_attn_b", [3 * D], f32, kind="ExternalInput").ap()
    cpw_d = nc.dram_tensor("c_proj_w", [D, D], bf16, kind="ExternalInput").ap()
    cpb_d = nc.dram_tensor("c_proj_b", [D], f32, kind="ExternalInput").ap()
    fcw_d = nc.dram_tensor("fc_w", [D, FF], bf16, kind="ExternalInput").ap()
    fcb_d = nc.dram_tensor("fc_b", [FF], f32, kind="ExternalInput").ap()
    pjw_d = nc.dram_tensor("proj_w", [FF, D], bf16, kind="ExternalInput").ap()
    pjb_d = nc.dram_tensor("proj_b", [D], f32, kind="ExternalInput").ap()
    out_d = nc.dram_tensor("out", [TPC, D], f32, kind="ExternalOutput").ap()

    PT = P * TPC  # elements in one [128, 512] slot region

    with tile.TileContext(nc) as tc:
        ctx = contextlib.ExitStack()
        with ctx:
            dram = ctx.enter_context(tc.tile_pool(name="dram", bufs=1,
                                                  space="DRAM"))
            consts = ctx.enter_context(tc.tile_pool(name="consts", bufs=1))
            wsl = ctx.enter_context(tc.tile_pool(name="wsl", bufs=9))
            resid = ctx.enter_context(tc.tile_pool(name="resid", bufs=1))
            acts = ctx.enter_context(tc.tile_pool(name="acts", bufs=1))
            attp = ctx.enter_context(tc.tile_pool(name="attp", bufs=1))
            temps = ctx.enter_context(tc.tile_pool(name="temps", bufs=4))
            stats = ctx.enter_context(tc.tile_pool(name="stats", bufs=2))
            psum = ctx.enter_context(tc.tile_pool(name="psum", bufs=1,
                                                  space="PSUM"))

            # a2a buffers (bf16); Q/K/V exchanged by three collectives, each
            # triggered as soon as its projection is evicted
            finQ = dram.tile([NC, PT], bf16)
            foutQ = dram.tile([NC, PT], bf16)
            finK = dram.tile([NC, PT], bf16)
            foutK = dram.tile([NC, PT], bf16)
            finV = dram.tile([NC, PT], bf16)
            foutV = dram.tile([NC, PT], bf16)
            SLOTB = (P + HPC) * TPC   # O^T plus one sums row per head
            a2a_bin = dram.tile([NC, SLOTB], bf16)
            a2a_bout = dram.tile([NC, SLOTB], bf16)

            # ---------------- x first: it heads the critical path ----------
            x_tiles = [resid.tile([P, D], f32, tag=f"x{t}", name=f"x{t}")
                       for t in range(TT)]
            xr = x_d.rearrange("(t p) d -> t p d", p=P)
            for t in range(TT):
                nc.sync.dma_start(x_tiles[t], xr[t])

            # ---------------- constants ----------------
            ident = consts.tile([P, P], bf16)
            make_identity(nc, ident)
            # mask[k, q] = 1 if q >= k (within a diagonal 128x128 strip)
            mask_f = consts.tile([P, P], f32)
            make_upper_triangular(nc, mask_f, val=1.0, diag=True)
            mask2 = consts.tile([P, HPC, P], bf16)
            for h in range(HPC):
                nc.vector.tensor_copy(out=mask2[:, h, :], in_=mask_f)

            # per-partition bias tile, pre-transposed on host (a strided DMA
            # here would emit thousands of 4-byte descriptors)
            bias_d = nc.dram_tensor("bias_pp", [P, 3 * DK + FFK], f32,
                                    kind="ExternalInput").ap()
            bias_pp = consts.tile([P, 3 * DK + FFK], f32)
            nc.sync.dma_start(bias_pp, bias_d)
            cab_qkv = bias_pp[:, 0:3 * DK]
            fcb_pp = bias_pp[:, 3 * DK:]

            def bcast_row(src_ap, off, n, name):
                tf = consts.tile([P, n], f32, name=name + "_f",
                                 tag="bcf", bufs=1)
                nc.sync.dma_start(tf, bass.AP(
                    tensor=src_ap.tensor, offset=src_ap.offset + off,
                    ap=[[0, P], [1, n]]))
                t = consts.tile([P, n], bf16, name=name)
                nc.vector.tensor_copy(out=t, in_=tf)
                return t

            # ---------------- weight slabs (shared rotating tag) ----------------
            # caw: 8 slabs [128, 3072]; cpw: 2 slabs [128, 4, 1024];
            # fcw: 8 m-slabs [128, 8, 512]; pjw: 8 slabs [128, 4, 1024].
            caw_r = caw_d.rearrange("(a p) f -> p a f", p=P)   # [128, 8, 3072]
            cpw_r = cpw_d.rearrange("(a p) d -> p a d", p=P)   # [128, 8, 1024]
            fcw_r = fcw_d.rearrange("(a p) f -> p a f", p=P)   # [128, 8, 4096]
            pjw_r = pjw_d.rearrange("(a p) d -> p a d", p=P)   # [128, 32, 1024]

            # c_attn weights column-grouped (Q slabs, then K, then V) so the
            # Q projection can start after only 2MB of weight DMA and each
            # group's collective fires as early as possible. The 2KB-slot tag
            # "w2k" is reused later by the MLP hidden tiles.
            caw_g = []
            for g in range(3):
                grp = []
                for k in range(DK):
                    t = wsl.tile([P, D], bf16, tag="w2k", bufs=24,
                                 name=f"caw{g}_{k}")
                    nc.gpsimd.dma_start(t, caw_r[:, k, g * D:(g + 1) * D])
                    grp.append(t)
                caw_g.append(grp)
            cawQ_sb, cawK_sb, cawV_sb = caw_g
            # cpw/fcw/pjw slab DMAs are emitted AFTER the forward collectives
            # on their queues so the collectives are not stuck behind their
            # slot-waits / descriptor generation.

            # ---------------- phase 1: LN1 + transpose ----------------
            def ln_transpose(dstT):
                """LayerNorm (ddof=1, eps on std) each [P, D] token tile of the
                residual, then PE-transpose into dstT [P, DK, TPC] (bf16)."""
                for t in range(TT):
                    xt = x_tiles[t]
                    st = stats.tile([P, 2, nc.vector.BN_STATS_DIM], f32,
                                    tag="bnst")
                    xg = xt.rearrange("p (g d) -> p g d", g=2)
                    for g in range(2):
                        nc.vector.bn_stats(out=st[:, g, :], in_=xg[:, g, :])
                    mv = stats.tile([P, nc.vector.BN_AGGR_DIM], f32, tag="mv")
                    nc.vector.bn_aggr(out=mv, in_=st)
                    sdev = stats.tile([P, 1], f32, tag="sdev")
                    nc.scalar.activation(out=sdev, in_=mv[:, 1:2], func=AF.Sqrt,
                                         scale=float(D) / (D - 1))
                    nc.vector.tensor_scalar_add(sdev, sdev, EPS)
                    rstd = stats.tile([P, 1], f32, tag="rstd")
                    nc.vector.reciprocal(out=rstd, in_=sdev)
                    nmr = stats.tile([P, 1], f32, tag="nmr")
                    nc.vector.tensor_scalar(out=nmr, in0=mv[:, 0:1],
                                            scalar1=rstd, scalar2=-1.0,
                                            op0=mybir.AluOpType.mult,
                                            op1=mybir.AluOpType.mult)
                    xn = temps.tile([P, D], bf16, tag="xn", bufs=2)
                    nc.scalar.activation(out=xn, in_=xt, func=AF.Identity,
                                         bias=nmr, scale=rstd)
                    for g in range(2):  # 4 transposes batched per eviction
                        pt = psum.tile([P, 4 * P], bf16, tag="ps", bufs=4,
                                       name="tp")
                        for i in range(4):
                            d = 4 * g + i
                            nc.tensor.transpose(pt[:, i * P:(i + 1) * P],
                                                xn[:, d * P:(d + 1) * P],
                                                ident)
                        nc.vector.tensor_copy(
                            out=dstT[:, 4 * g:4 * g + 4, t * P:(t + 1) * P],
                            in_=pt.rearrange("p (i c) -> p i c", c=P))

            xnT = acts.tile([P, DK, TPC], bf16, tag="xnT", name="xnT")
            ln_transpose(xnT)

            # -------- phase 2: Q, K, V^T projections, one a2a each --------
            for half, (wsb, fin, fout) in enumerate(
                    [(cawQ_sb, finQ, foutQ), (cawK_sb, finK, foutK),
                     (cawV_sb, finV, foutV)]):
                for j in range(DK):  # 8 feature tiles each
                    m = half * DK + j
                    ps = psum.tile([P, TPC], f32, tag="ps", bufs=4,
                                   name="mmps")
                    for k in range(DK):
                        nc.tensor.matmul(ps, wsb[k][:, j * P:(j + 1) * P],
                                         xnT[:, k, :],
                                         start=(k == 0), stop=(k == DK - 1))
                    sb = temps.tile([P, TPC], bf16, tag="ev", bufs=2,
                                    name="ev")
                    nc.scalar.activation(out=sb, in_=ps, func=AF.Identity,
                                         bias=cab_qkv[:, m:m + 1])
                    nc.sync.dma_start(
                        fin[j, :].rearrange("(a b) -> a b", b=TPC), sb)
                nc.gpsimd.collective_compute(
                    "AllToAll", mybir.AluOpType.bypass,
                    replica_groups=[list(range(NC))],
                    ins=[fin.opt()], outs=[fout.opt()])

            # cpw/fcw slabs: issued on gpsimd after the forward collectives
            # (slots are free once QKV released the caw slabs; the bwd
            # collective behind them is not needed until attention ends).
            cpw_sb = []
            for c in range(2):
                t = wsl.tile([P, 4, D], bf16, tag="wsl", name=f"cpw{c}")
                nc.gpsimd.dma_start(t, cpw_r[:, 4 * c:4 * c + 4, :])
                cpw_sb.append(t)
            fcw_sb = []
            for g in range(NBLK):
                t = wsl.tile([P, DK, 512], bf16, tag="wsl", name=f"fcw{g}")
                nc.gpsimd.dma_start(t, fcw_r[:, :, g * 512:(g + 1) * 512])
                fcw_sb.append(t)

            # ---------------- phase 4: attention (my 2 heads, all tokens) ----
            # K^T resident: [128 feat, slot, 512 tok]
            ktr = attp.tile([P, NC, TPC], bf16, tag="ktr", name="ktr")
            nc.sync.dma_start(
                ktr, foutK.rearrange("n (p t) -> p n t", p=P))
            # V resident with ones col per head: [128 tok, slot, t, 2*(64+1)]
            # foutV lands contiguously in vtmp (big DMA descriptors); a vector
            # repack inserts the per-head layout (tiny DMA descriptors would
            # cost ~25us otherwise).
            vres = attp.tile([P, NC, TT, HPC * (HD + 1)], bf16, tag="vres",
                             name="vres")
            for j in range(NC):
                nc.vector.memset(
                    vres[:, j, :, :].rearrange("p t (h c) -> p t h c",
                                               c=HD + 1)[:, :, :, HD:HD + 1],
                    1.0)
                vtr = attp.tile([P, TPC], bf16, tag="vtr", bufs=2,
                                name="vtr")
                nc.sync.dma_start(
                    vtr, foutV[j, :].rearrange("(p t) -> p t", p=P))
                for t in range(TT):
                    pt = psum.tile([P, P], bf16, tag="ps", bufs=4, name="vtp")
                    nc.tensor.transpose(pt, vtr[:, t * P:(t + 1) * P], ident)
                    nc.vector.tensor_copy(
                        out=vres[:, j, t, :].rearrange(
                            "p (h c) -> p h c", c=HD + 1)[:, :, 0:HD],
                        in_=pt.rearrange("p (h c) -> p h c", c=HD))

            isc = 1.0 / float(np.sqrt(HD))
            for b in range(B):
                for qb in range(NQB):
                    slot_q = b * NQB + qb
                    qt = attp.tile([P, QB], bf16, tag="qt", bufs=2, name="qt")
                    nc.sync.dma_start(
                        qt, foutQ[slot_q, :].rearrange("(a b) -> a b",
                                                           b=TPC))
                    op = psum.tile([HD + 1, HPC * QB], f32, tag="ps", bufs=4,
                                   name="op")
                    nkt = (qb + 1) * KPB
                    for kt in range(nkt):
                        src_slot = b * NQB + kt // KPB
                        off = (kt % KPB) * P
                        d = kt - qb * KPB  # >= 0 on diagonal strips
                        qs = max(d, 0) * P
                        sps = psum.tile([P, HPC * QB], f32, tag="ps", bufs=4,
                                        name="sps")
                        for h in range(HPC):
                            nc.tensor.matmul(
                                sps[:, h * QB + qs:(h + 1) * QB],
                                ktr[h * HD:(h + 1) * HD, src_slot,
                                    off:off + P],
                                qt[h * HD:(h + 1) * HD, qs:QB],
                                start=True, stop=True)
                        at = attp.tile([P, HPC, QB], bf16, tag="at", bufs=3,
                                       name="at")
                        spsv = sps.rearrange("p (h q) -> p h q", h=HPC)
                        nc.scalar.activation(out=at[:, :, qs:QB],
                                             in_=spsv[:, :, qs:QB],
                                             func=AF.Exp, scale=isc)
                        if d >= 0:
                            nc.vector.tensor_mul(out=at[:, :, qs:qs + P],
                                                 in0=at[:, :, qs:qs + P],
                                                 in1=mask2)
                        for h in range(HPC):
                            nc.tensor.matmul(
                                op[:, h * QB + qs:(h + 1) * QB],
                                vres[:, src_slot, kt % KPB,
                                     h * (HD + 1):(h + 1) * (HD + 1)],
                                at[:, h, qs:QB],
                                start=(kt == 0), stop=(kt == nkt - 1))
                    # normalize: recip of the sums row, partition-broadcast
                    # via a cheap PE matmul (ones64^T @ rc), O^T copied to
                    # SBUF so the multiply has a single PSUM operand
                    # ship unnormalized O^T plus the sums rows; the receiver
                    # normalizes after the bwd a2a (keeps the q-block tail to
                    # one copy + DMAs: no reciprocal, no act-table churn)
                    otv = attp.tile([HD + 1, HPC * QB], bf16, tag="otv",
                                    bufs=2, name="otv")
                    nc.vector.tensor_copy(out=otv, in_=op[0:HD + 1, :])
                    for h in range(HPC):
                        nc.sync.dma_start(
                            a2a_bin[slot_q, h * HD * TPC:(h + 1) * HD * TPC]
                            .rearrange("(a b) -> a b", b=TPC),
                            otv[0:HD, h * QB:(h + 1) * QB])
                        nc.sync.dma_start(
                            a2a_bin[slot_q, (P + h) * TPC:(P + h + 1) * TPC]
                            .rearrange("(a b) -> a b", b=TPC),
                            otv[HD:HD + 1, h * QB:(h + 1) * QB])

            # ---------------- phase 3c: backward AllToAll (O^T) ----------------
            nc.gpsimd.collective_compute(
                "AllToAll", mybir.AluOpType.bypass,
                replica_groups=[list(range(NC))],
                ins=[a2a_bin.opt()], outs=[a2a_bout.opt()])

            # ---------------- phase 5: attn out-proj + residual (in place) ----
            otf = acts.tile([P, DK, TPC], bf16, tag="xnT", name="otf")
            nc.sync.dma_start(otf, a2a_bout[:, 0:PT].rearrange(
                "n (p t) -> p n t", p=P))
            # receiver-side softmax normalization: one packed reciprocal over
            # all 16 sums rows, then per-slot partition-broadcasts + muls
            sr = attp.tile([P, NC * DK], bf16, tag="sr", name="sr")
            for j in range(NC):
                nc.sync.dma_start(
                    sr[:, DK * j:DK * (j + 1)],
                    a2a_bout[j, PT:PT + HPC * TPC].rearrange(
                        "(p m) -> p m", m=DK))
            srf = attp.tile([P, NC * DK], f32, tag="srf", name="srf")
            nc.vector.reciprocal(out=srf, in_=sr)
            srd = dram.tile([NC, HPC * TPC], f32, tag="srd", name="srd")
            for j in range(NC):
                nc.sync.dma_start(
                    srd[j, :].rearrange("(p m) -> p m", m=DK),
                    srf[:, DK * j:DK * (j + 1)])
            for j in range(NC):
                rbc = attp.tile([P, TPC], f32, tag="rbc", bufs=2, name="rbc")
                for h in range(HPC):
                    nc.sync.dma_start(
                        rbc[h * HD:(h + 1) * HD, :], bass.AP(
                            tensor=srd.tensor,
                            offset=srd.offset + j * HPC * TPC + h * TPC,
                            ap=[[0, HD], [1, TPC]]))
                nc.vector.tensor_mul(out=otf[:, j, :], in0=otf[:, j, :],
                                     in1=rbc)
            # pjw slabs on the sync queue here: the only sync work emitted
            # after them is the final out DMA, so their slot-waits (on caw/cpw/
            # fcw slab releases, all PE-driven) cannot head-of-line block any
            # compute. Slabs 0-4 bind to slots already free by this point;
            # 5-7 bind as the first fc blocks release their fcw slabs.
            pjw_sb = []
            for g in range(NBLK):
                t = wsl.tile([P, 4, D], bf16, tag="wsl", name=f"pjw{g}")
                nc.sync.dma_start(t, pjw_r[:, 4 * g:4 * g + 4, :])
                pjw_sb.append(t)
            cpb_bc = bcast_row(cpb_d, 0, D, "cpb_bc")
            pjb_bc = bcast_row(pjb_d, 0, D, "pjb_bc")
            for t in range(TT):
                nc.vector.tensor_add(out=x_tiles[t], in0=x_tiles[t],
                                     in1=cpb_bc)
            for t in range(TT):
                ps = psum.tile([P, D], f32, tag="ps", bufs=4, name="mmps")
                for nb in range(2):
                    ns = D // 2
                    for k in range(DK):
                        nc.tensor.matmul(
                            ps[:, nb * ns:(nb + 1) * ns],
                            otf[:, k, t * P:(t + 1) * P],
                            cpw_sb[k // 4][:, k % 4, nb * ns:(nb + 1) * ns],
                            start=(k == 0), stop=(k == DK - 1))
                nc.vector.tensor_add(out=x_tiles[t], in0=ps, in1=x_tiles[t])

            # ---------------- phase 6: LN2 + transpose ----------------
            xn2T = acts.tile([P, DK, TPC], bf16, tag="xnT", name="xn2T")
            ln_transpose(xn2T)

            # ---------------- phase 7+8: fused MLP blocks ----------------
            for t in range(TT):
                nc.vector.tensor_add(out=x_tiles[t], in0=x_tiles[t],
                                     in1=pjb_bc)
            for g in range(NBLK):
                ht = []
                for mm in range(4):
                    m = g * 4 + mm
                    ps = psum.tile([P, TPC], f32, tag="ps", bufs=4,
                                   name="mmps")
                    for k in range(DK):
                        nc.tensor.matmul(
                            ps, fcw_sb[g][:, k, mm * P:(mm + 1) * P],
                            xn2T[:, k, :],
                            start=(k == 0), stop=(k == DK - 1))
                    hm = wsl.tile([P, TPC], bf16, tag="w2k", bufs=24,
                                  name="hT")
                    nc.scalar.activation(out=hm, in_=ps,
                                         func=AF.Gelu_apprx_tanh,
                                         bias=fcb_pp[:, m:m + 1])
                    ht.append(hm)
                for t in range(TT):
                    ps = psum.tile([P, D], f32, tag="ps", bufs=4,
                                   name="mmps")
                    for nb in range(2):
                        ns = D // 2
                        for kk in range(4):
                            nc.tensor.matmul(
                                ps[:, nb * ns:(nb + 1) * ns],
                                ht[kk][:, t * P:(t + 1) * P],
                                pjw_sb[g][:, kk, nb * ns:(nb + 1) * ns],
                                start=(kk == 0), stop=(kk == 3))
                    nc.vector.tensor_add(out=x_tiles[t], in0=ps,
                                         in1=x_tiles[t])

            # ---------------- output ----------------
            outr = out_d.rearrange("(t p) d -> t p d", p=P)
            for t in range(TT):
                nc.sync.dma_start(outr[t], x_tiles[t])

    nc.compile()
    return nc


_NC_CACHE = None


def _get_program():
    global _NC_CACHE
    if _NC_CACHE is None:
        _NC_CACHE = build_program()
    return _NC_CACHE


def host_fold(inputs):
    """Fold LN scale/bias into the following matmul weights; cast weights to
    bf16 (host side)."""
    import ml_dtypes
    bf = ml_dtypes.bfloat16

    def f(a):
        return np.ascontiguousarray(np.asarray(a), dtype=np.float32)
    x = f(inputs["x"]).reshape(B * S, D)
    caw0 = f(inputs["c_attn_w"])
    fcw0 = f(inputs["fc_w"])
    caw = caw0 * f(inputs["ln1_w"])[:, None]
    cab = f(inputs["c_attn_b"]) + f(inputs["ln1_b"]) @ caw0
    fcw = fcw0 * f(inputs["ln2_w"])[:, None]
    fcb = f(inputs["fc_b"]) + f(inputs["ln2_b"]) @ fcw0

    def w(a):
        return np.ascontiguousarray(np.asarray(a, dtype=bf))
    # per-partition bias tile [128, 56]: Q,K,V biases then fc biases
    bias_pp = np.concatenate([cab.reshape(3 * DK, P).T,
                              fcb.reshape(FFK, P).T], axis=1)
    return {
        "x": x,
        "c_attn_w": w(caw), "c_attn_b": f(cab),
        "c_proj_w": w(inputs["c_proj_w"]), "c_proj_b": f(inputs["c_proj_b"]),
        "fc_w": w(fcw), "fc_b": f(fcb),
        "proj_w": w(inputs["proj_w"]), "proj_b": f(inputs["proj_b"]),
        "bias_pp": np.ascontiguousarray(bias_pp, dtype=np.float32),
    }


def make_in_maps(inputs):
    full = host_fold(inputs)
    in_maps = []
    for c in range(NC):
        m = dict(full)
        m["x"] = np.ascontiguousarray(full["x"][c * TPC:(c + 1) * TPC])
        in_maps.append(m)
    return in_maps


def kernel(**inputs) -> np.ndarray:
    from concourse import bass_utils
    nc = _get_program()
    in_maps = make_in_maps(inputs)
    res = bass_utils.run_bass_kernel_spmd(nc, in_maps, core_ids=list(range(NC)))
    out = np.concatenate([res.results[c]["out"] for c in range(NC)], axis=0)
    return out.reshape(B, S, D)
